# revision 10
# baseline (speedup 1.0000x reference)
"""Bass/Tile TRN2 kernel for nn_MultiHeadAttention_9277129359942.

B=2, T=S=2048, D=1024, H=16 heads, head_dim=64, fp32 I/O.

Sharding (8 cores): data-parallel over batch (2) x tensor-parallel over
head groups (4 heads / core, 256 out dims).  Each core computes the
attention for its 4 heads and a partial output projection; the host sums
the 4 bf16 partials per batch and adds the (linear) bo and bv terms
exactly: out = sum_g partial_g + bo + bv @ Wo.T.

v3 design notes:
  - Softmax exp is split across engines: head A of each pair uses the
    exact ACT exp (2 x N=512 chunks, started as soon as each score
    chunk lands), head B uses a one-instruction DVE fast-exp
    (Schraudolph: int16(x*EA+EC) bitcast as bf16, ~4% max rel err).
    End-to-end rel err 1.39e-2 (gate 2e-2), verified vs the reference.
  - Software pipelining in the attention loop: ctxA (ACT head) deferred
    one s-iteration; ctxB chunk0 same-iteration, chunk1 next iteration.
    Steady-state period ~1.5us/iter with PE/ACT/DVE all ~90% busy.
    PSUM: scA 2 + scB 2 + ctxA 2 + ctxB 2 = 8 banks.
  - Softmax denominators: the ones-column of v_aug makes row 64 of each
    ctx psum the denominator; 1/x runs on a [128,16] reshape (DVE
    reciprocal is ~6 cyc/elem per LANE, so a [1,1024] row costs 6.5us
    but [128,16] costs ~0.1us); the 64-partition broadcast is a log2
    SBUF DMA chain (last block: K=1 PE matmul so the tail never waits).
  - Normalize multiplies run on GpSimd (SBUF-only engine, otherwise
    idle); psum evictions and out-proj drains alternate DVE/ACT.
  - Inputs are DMA'd as full 128-partition tiles (engages all 16 SDMA
    engines) split across the sync and scalar HWDGE queues; q/k tiles
    first so the projections chase the loads.  Output is bf16 (halves
    the tail DMA); bo/bv are applied on the host (linear terms).
"""

import os
import sys

import numpy as np

for _p in ("/opt/trn_rl_repo",):
    if os.path.isdir(_p) and _p not in sys.path:
        sys.path.append(_p)

import ml_dtypes

import concourse.bass as bass
import concourse.mybir as mybir
import concourse.tile as tile
from concourse import bacc
from concourse.bass_utils import run_bass_kernel_spmd

F32 = mybir.dt.float32
BF16 = mybir.dt.bfloat16
I16 = mybir.dt.int16
AF = mybir.ActivationFunctionType
ALU = mybir.AluOpType
BF16_NP = ml_dtypes.bfloat16

D = 1024          # model dim
T = 2048          # query length
S = 2048          # key length
P = 128           # partitions
KT = D // P       # 8 contraction tiles
TT = T // P       # 16 row tiles
ST = S // P       # 16 key tiles
HL = 4            # local heads per core
HD = 64           # head dim
OUTL = HL * HD    # 256 local out dims
VW = HD + 1       # v_aug width per head (ones column appended)
N_CORES = 8

# fast-exp constants: exp(x*0.125) ~= bf16(bitcast(int16(x*EA + EC)))
EA = float(0.125 * 128.0 / np.log(2.0))
EC = float(127 * 128 - 7.5)
I32 = mybir.dt.int32
RMAGIC = 0x7EF311C3   # int-trick reciprocal seed constant


def build_program():
    """Build + compile the SPMD program (same on all 8 cores)."""
    nc = bacc.Bacc(
        "TRN2", target_bir_lowering=False, debug=False, enable_asserts=True,
        num_devices=N_CORES,
    )

    xq_d = nc.dram_tensor("xq", [D, T], BF16, kind="ExternalInput")
    xk_d = nc.dram_tensor("xk", [D, S], BF16, kind="ExternalInput")
    xv_d = nc.dram_tensor("xv", [D, S], BF16, kind="ExternalInput")
    wq_d = nc.dram_tensor("wq", [D, OUTL], BF16, kind="ExternalInput")
    wk_d = nc.dram_tensor("wk", [D, OUTL], BF16, kind="ExternalInput")
    wv_d = nc.dram_tensor("wv", [D, OUTL], BF16, kind="ExternalInput")
    wo_d = nc.dram_tensor("wo", [OUTL, D], BF16, kind="ExternalInput")
    bq_d = nc.dram_tensor("bq", [OUTL, 1], F32, kind="ExternalInput")
    bk_d = nc.dram_tensor("bk", [OUTL, 1], F32, kind="ExternalInput")
    out_d = nc.dram_tensor("out", [T, D], BF16, kind="ExternalOutput")
    wsink_d = nc.dram_tensor("warm_sink", [1, 8], F32, kind="ExternalOutput")

    with tile.TileContext(nc) as tc:
        _build(nc, tc, xq_d, xk_d, xv_d, wq_d, wk_d, wv_d, wo_d,
               bq_d, bk_d, out_d, wsink_d)
    nc.compile()
    return nc


def _build(nc, tc, xq_d, xk_d, xv_d, wq_d, wk_d, wv_d, wo_d,
           bq_d, bk_d, out_d, wsink_d):
    from contextlib import ExitStack

    stack = ExitStack()
    with stack:
        consts = stack.enter_context(tc.tile_pool(name="consts", bufs=1))
        wpool = stack.enter_context(tc.tile_pool(name="wpool", bufs=1))
        acts = stack.enter_context(tc.tile_pool(name="acts", bufs=1))
        stgpool = stack.enter_context(tc.tile_pool(name="stgpool", bufs=1))
        nrmpool = stack.enter_context(tc.tile_pool(name="nrmpool", bufs=1))

        bq_sb = consts.tile([P, 2], F32, name="bq", tag="bq")
        bk_sb = consts.tile([P, 2], F32, name="bk", tag="bk")
        wsnk = consts.tile([1, 8], F32, name="wsnk", tag="wsnk")

        wq_sb = [wpool.tile([P, OUTL], BF16, name=f"wq{k}", tag=f"wq{k}")
                 for k in range(KT)]
        wk_sb = [wpool.tile([P, OUTL], BF16, name=f"wk{k}", tag=f"wk{k}")
                 for k in range(KT)]
        wv_sb = [wpool.tile([P, OUTL], BF16, name=f"wv{k}", tag=f"wv{k}")
                 for k in range(KT)]
        wo_sb = [wpool.tile([P, D], BF16, name=f"wo{k}", tag=f"wo{k}")
                 for k in range(2)]

        qT = [acts.tile([P, T], BF16, name=f"qT{m}", tag=f"qT{m}")
              for m in range(2)]
        kT = [acts.tile([P, S], BF16, name=f"kT{m}", tag=f"kT{m}")
              for m in range(2)]
        v_aug = acts.tile([P, ST * HL * VW], BF16, name="vaug", tag="vaug")
        ctxT = [[acts.tile([P, 1024], BF16, name=f"ctxT{g}{th}",
                           tag=f"ctxT{g}{th}") for th in range(2)]
                for g in range(2)]

        xpool = stack.enter_context(tc.tile_pool(name="xpool", bufs=1))
        xq_sb = [xpool.tile([P, T], BF16, name=f"xq{k}", tag=f"xq{k}")
                 for k in range(KT)]
        xk_sb = [xpool.tile([P, S], BF16, name=f"xk{k}", tag=f"xk{k}")
                 for k in range(KT)]
        xv_sb = [xpool.tile([P, S], BF16, name=f"xv{k}", tag=f"xv{k}")
                 for k in range(KT)]

        # Full 128-partition DMAs (all 16 SDMA engines per transfer),
        # alternating sync/scalar queues; q/k first, v later, wo last.
        nc.sync.dma_start(wq_sb[0][:], wq_d[0:P, :])
        nc.scalar.dma_start(wk_sb[0][:], wk_d[0:P, :])
        nc.sync.dma_start(bq_sb[:], bq_d.rearrange("(m p) o -> p (m o)", p=P))
        nc.scalar.dma_start(bk_sb[:], bk_d.rearrange("(m p) o -> p (m o)",
                                                     p=P))
        nc.sync.dma_start(xq_sb[0][:], xq_d[0:P, :])
        nc.scalar.dma_start(xk_sb[0][:], xk_d[0:P, :])
        for k in range(1, KT):
            nc.sync.dma_start(wq_sb[k][:], wq_d[k * P:(k + 1) * P, :])
            nc.scalar.dma_start(wk_sb[k][:], wk_d[k * P:(k + 1) * P, :])
            nc.sync.dma_start(xq_sb[k][:], xq_d[k * P:(k + 1) * P, :])
            nc.scalar.dma_start(xk_sb[k][:], xk_d[k * P:(k + 1) * P, :])
        for k in range(KT):
            eng = nc.sync if k % 2 == 0 else nc.scalar
            eng.dma_start(wv_sb[k][:], wv_d[k * P:(k + 1) * P, :])
        for k in range(KT):
            eng = nc.sync if k % 2 == 0 else nc.scalar
            eng.dma_start(xv_sb[k][:], xv_d[k * P:(k + 1) * P, :])
        for k in range(2):
            eng = nc.sync if k % 2 == 0 else nc.scalar
            eng.dma_start(wo_sb[k][:], wo_d[k * P:(k + 1) * P, :])

        nc.vector.memset(v_aug[:], 1.0)  # ones columns survive the v writes
        twos = consts.tile([1, 1024], F32, name="twos", tag="twos")
        nc.vector.memset(twos[:], 2.0)

        # ACT exp-table preload during the DMA head
        nc.scalar.activation(wsnk[0:1, 0:2], bq_sb[0:1, 0:2], AF.Exp)

        # ---- q/k projections -------------------------------------------
        with tc.tile_pool(name="qkpsum", bufs=2, space="PSUM") as qkpsum:
            # PE warmup burst (HAM un-throttle) during the DMA head
            warm = qkpsum.tile([P, T], F32, name="pqk", tag="pqk")
            for w in range(16):
                nc.tensor.matmul(warm[:, 0:OUTL], wq_sb[0][:, 0:P],
                                 wq_sb[0][:], start=(w == 0), stop=(w == 15))
            nc.vector.tensor_copy(wsnk[0:1, 2:4], warm[0:1, 0:2])
            nc.sync.dma_start(wsink_d[:, :], wsnk[:])

            for m in range(2):
                for w_sb, x_sb, b_sb, o_sb in (
                        (wq_sb, xq_sb, bq_sb, qT),
                        (wk_sb, xk_sb, bk_sb, kT)):
                    ps = qkpsum.tile([P, T], F32, name="pqk", tag="pqk")
                    for k in range(KT):
                        for c in range(4):
                            cs = slice(c * 512, (c + 1) * 512)
                            nc.tensor.matmul(
                                ps[:, cs], w_sb[k][:, m * P:(m + 1) * P],
                                x_sb[k][:, cs],
                                start=(k == 0), stop=(k == KT - 1))
                    nc.scalar.activation(o_sb[m][:], ps[:], AF.Identity,
                                         bias=b_sb[:, m:m + 1])

        # ---- v projection (no bias: bv is applied on the host) ---------
        with tc.tile_pool(name="vpsum", bufs=2, space="PSUM") as vpsum:
            for s in range(ST):
                ps = vpsum.tile([P, OUTL], F32, name="pv", tag="pv")
                for k in range(KT):
                    nc.tensor.matmul(
                        ps[:], xv_sb[k][:, s * P:(s + 1) * P], wv_sb[k][:],
                        start=(k == 0), stop=(k == KT - 1))
                dst = v_aug[:, s * HL * VW:(s + 1) * HL * VW]
                dst = dst.rearrange("p (h x) -> p h x", x=VW)[:, :, 0:HD]
                nc.scalar.copy(dst, ps[:].rearrange("p (h x) -> p h x", x=HD))

        # ---- attention: 4 blocks of (head pair p, t-half th) -----------
        with tc.tile_pool(name="scpsum", bufs=1, space="PSUM") as scpsum, \
             tc.tile_pool(name="ctxpsum", bufs=1, space="PSUM") as ctxpsum, \
             tc.tile_pool(name="epool", bufs=2) as epool:

            for p in range(2):          # head pair (local heads 2p, 2p+1)
                for th in range(2):     # t halves of 1024
                    t0 = th * 1024
                    ctxA = ctxpsum.tile([VW, 1024], F32, name="ctxA",
                                        tag="ctxA")
                    ctxB = ctxpsum.tile([VW, 1024], F32, name="ctxB",
                                        tag="ctxB")
                    hA = 2 * p
                    hB = 2 * p + 1

                    def vslice(h, s):
                        return slice(s * HL * VW + h * VW,
                                     s * HL * VW + (h + 1) * VW)

                    prevA = None   # (s, exA)
                    prevB = None   # (s, eiB)
                    for s in range(ST):
                        ss = slice(s * P, (s + 1) * P)
                        scA = scpsum.tile([P, 1024], F32, name="scA",
                                          tag="scA")
                        scB = scpsum.tile([P, 1024], F32, name="scB",
                                          tag="scB")
                        for c in range(2):
                            cs = slice(c * 512, (c + 1) * 512)
                            ts_ = slice(t0 + c * 512, t0 + (c + 1) * 512)
                            nc.tensor.matmul(scA[:, cs], kT[p][0:HD, ss],
                                             qT[p][0:HD, ts_],
                                             start=True, stop=True)
                            nc.tensor.matmul(scB[:, cs], kT[p][HD:P, ss],
                                             qT[p][HD:P, ts_],
                                             start=True, stop=True)
                        exA = epool.tile([P, 1024], BF16, name="exA",
                                         tag="exA")
                        eiB = epool.tile([P, 1024], I16, name="eiB",
                                         tag="eiB")
                        # DVE fast-exp, one instruction (per-instr overhead
                        # on the DVE is ~0.4us, so merged beats chunked)
                        nc.vector.tensor_scalar(eiB[:], scB[:],
                                                EA, EC, op0=ALU.mult,
                                                op1=ALU.add)
                        # ACT exact exp, chunked (starts after score chunk0)
                        nc.scalar.activation(exA[:, 0:512], scA[:, 0:512],
                                             AF.Exp, scale=0.125)
                        nc.scalar.activation(exA[:, 512:1024],
                                             scA[:, 512:1024],
                                             AF.Exp, scale=0.125)
                        # deferred ctxA/ctxB for iteration s-1
                        if prevA is not None:
                            sp, pexA = prevA
                            for c in range(2):
                                cs = slice(c * 512, (c + 1) * 512)
                                nc.tensor.matmul(
                                    ctxA[:, cs], v_aug[:, vslice(hA, sp)],
                                    pexA[:, cs],
                                    start=(sp == 0), stop=(sp == ST - 1))
                            sp, peiB = prevB
                            ebB = peiB[:].bitcast(BF16)
                            for c in range(2):
                                cs = slice(c * 512, (c + 1) * 512)
                                nc.tensor.matmul(
                                    ctxB[:, cs], v_aug[:, vslice(hB, sp)],
                                    ebB[:, cs],
                                    start=(sp == 0), stop=(sp == ST - 1))
                        prevA = (s, exA)
                        prevB = (s, eiB)
                    # drain the deferred tails
                    sp, pexA = prevA
                    sp2, peiB = prevB
                    for c in range(2):
                        cs = slice(c * 512, (c + 1) * 512)
                        nc.tensor.matmul(ctxA[:, cs],
                                         v_aug[:, vslice(hA, sp)],
                                         pexA[:, cs],
                                         start=(sp == 0), stop=True)
                    ebB = peiB[:].bitcast(BF16)
                    for c in range(2):
                        cs = slice(c * 512, (c + 1) * 512)
                        nc.tensor.matmul(ctxB[:, cs],
                                         v_aug[:, vslice(hB, sp2)],
                                         ebB[:, cs],
                                         start=(sp2 == 0), stop=True)

                    # evict ctx (DVE + ACT) and denom rows (DVE, to base
                    # partition 0); 1/denom = int-trick seed (DVE) + two
                    # Newton steps, broadcast, and normalize multiplies all
                    # on the otherwise idle GpSimd engine.
                    stgA = stgpool.tile([HD, 1024], F32, name="stgA",
                                        tag="stgA")
                    stgB = stgpool.tile([HD, 1024], F32, name="stgB",
                                        tag="stgB")
                    nc.vector.tensor_copy(stgA[:], ctxA[0:HD, :])
                    nc.scalar.copy(stgB[:], ctxB[0:HD, :])
                    for i, (ctx, stg) in ((0, (ctxA, stgA)),
                                          (1, (ctxB, stgB))):
                        drow = nrmpool.tile([1, 1024], F32, name=f"dr{i}",
                                            tag="dr")
                        nc.vector.tensor_copy(drow[:], ctx[HD:HD + 1, :])
                        seed = nrmpool.tile([1, 1024], I32, name=f"sd{i}",
                                            tag="sd")
                        nc.vector.tensor_scalar(
                            seed[:], drow[:].bitcast(I32), -1, RMAGIC,
                            op0=ALU.mult, op1=ALU.add)
                        r = seed[:].bitcast(F32)
                        for it in range(2):
                            e = nrmpool.tile([1, 1024], F32,
                                             name=f"e{i}{it}", tag="e")
                            nc.gpsimd.tensor_tensor(out=e[:], in0=drow[:],
                                                    in1=r, op=ALU.mult)
                            t_ = nrmpool.tile([1, 1024], F32,
                                              name=f"t{i}{it}", tag="t")
                            nc.gpsimd.tensor_tensor(out=t_[:], in0=twos[:],
                                                    in1=e[:],
                                                    op=ALU.subtract)
                            rn = nrmpool.tile([1, 1024], F32,
                                              name=f"rn{i}{it}",
                                              tag=f"rn{it}")
                            nc.gpsimd.tensor_tensor(out=rn[:], in0=r,
                                                    in1=t_[:], op=ALU.mult)
                            r = rn[:]
                        rb = nrmpool.tile([HD, 1024], F32,
                                          name=f"rb{i}", tag="rb")
                        nc.gpsimd.partition_broadcast(rb[:], r)
                        if i == 0:
                            nc.gpsimd.tensor_tensor(
                                out=ctxT[p][th][0:HD, :],
                                in0=stg[:], in1=rb[:],
                                op=ALU.mult)
                        else:
                            ostg = nrmpool.tile([HD, 1024], BF16,
                                                name="ostg", tag="ostg")
                            nc.gpsimd.tensor_tensor(
                                out=ostg[:], in0=stg[:],
                                in1=rb[:], op=ALU.mult)
                            nc.scalar.dma_start(
                                ctxT[p][th][HD:P, :], ostg[:])

        # ---- output projection -----------------------------------------
        with tc.tile_pool(name="popsum", bufs=2, space="PSUM") as popsum, \
             tc.tile_pool(name="opool", bufs=3) as opool:

            def emit_outproj(trange):
                for t in trange:
                    th_, tt_ = divmod(t, TT // 2)
                    ts_ = slice(tt_ * P, (tt_ + 1) * P)
                    po = popsum.tile([P, D], F32, name="po", tag="po")
                    for g in range(2):
                        for n in range(2):
                            ns = slice(n * 512, (n + 1) * 512)
                            nc.tensor.matmul(po[:, ns],
                                             ctxT[g][th_][:, ts_],
                                             wo_sb[g][:, ns],
                                             start=(g == 0), stop=(g == 1))
                    ost = opool.tile([P, D], BF16, name="ost", tag="ost")
                    if t % 2 == 0:
                        nc.vector.tensor_copy(ost[:], po[:])
                    else:
                        nc.scalar.copy(ost[:], po[:])
                    nc.sync.dma_start(out_d[t * P:(t + 1) * P, :], ost[:])

            emit_outproj(range(0, TT))


def make_in_maps(query, key, value, Wq, bq, Wk, bk, Wv, bv, Wo, bo):
    """Shard the full inputs into the 8 per-core input dicts."""
    query, key, value, Wq, bq, Wk, bk, Wv, bv, Wo, bo = [
        np.asarray(a, dtype=np.float32)
        for a in (query, key, value, Wq, bq, Wk, bk, Wv, bv, Wo, bo)]

    def bf(a):
        return np.ascontiguousarray(a).astype(BF16_NP)

    in_maps = []
    for c in range(N_CORES):
        b, g = divmod(c, 4)
        sl = slice(g * OUTL, (g + 1) * OUTL)
        in_maps.append({
            "xq": bf(query[b].T),
            "xk": bf(key[b].T),
            "xv": bf(value[b].T),
            "wq": bf(Wq[sl, :].T),
            "wk": bf(Wk[sl, :].T),
            "wv": bf(Wv[sl, :].T),
            "wo": bf(Wo[:, sl].T),
            "bq": np.ascontiguousarray(bq[sl].reshape(OUTL, 1)),
            "bk": np.ascontiguousarray(bk[sl].reshape(OUTL, 1)),
        })
    return in_maps


def gather_out(results, Wo, bo, bv):
    """Sum the per-core bf16 partials and add the host-side bias terms."""
    Wo = np.asarray(Wo, np.float32)
    bo = np.asarray(bo, np.float32)
    bv = np.asarray(bv, np.float32)
    host_bias = bo + bv @ Wo.T
    out = np.empty((2, T, D), dtype=np.float32)
    for b in range(2):
        acc = results[4 * b]["out"].astype(np.float32)
        for g in range(1, 4):
            acc = acc + results[4 * b + g]["out"].astype(np.float32)
        out[b] = acc + host_bias
    return out


_NC_CACHE = None


def _get_nc():
    global _NC_CACHE
    if _NC_CACHE is None:
        _NC_CACHE = build_program()
    return _NC_CACHE


def kernel(query, key, value, Wq, bq, Wk, bk, Wv, bv, Wo, bo):
    nc = _get_nc()
    in_maps = make_in_maps(query, key, value, Wq, bq, Wk, bk, Wv, bv, Wo, bo)
    res = run_bass_kernel_spmd(nc, in_maps, list(range(N_CORES))).results
    return gather_out(res, Wo, bo, bv)


# revision 11
# speedup vs baseline: 1.2958x; 1.2958x over previous
"""Bass/Tile TRN2 kernel for nn_MultiHeadAttention_9277129359942.

B=2, T=S=2048, D=1024, H=16 heads, head_dim=64, fp32 I/O.

Sharding (8 cores): data-parallel over batch (2) x tensor-parallel over
head groups (4 heads / core, 256 out dims).  Each core computes the
attention for its 4 heads and a partial output projection; the host sums
the 4 bf16 partials per batch and adds the (linear) bo and bv terms
exactly: out = sum_g partial_g + bo + bv @ Wo.T.

v3 design notes:
  - Softmax exp is split across engines: head A of each pair uses the
    exact ACT exp (2 x N=512 chunks, started as soon as each score
    chunk lands), head B uses a one-instruction DVE fast-exp
    (Schraudolph: int16(x*EA+EC) bitcast as bf16, ~4% max rel err).
    End-to-end rel err 1.39e-2 (gate 2e-2), verified vs the reference.
  - Software pipelining in the attention loop: ctxA (ACT head) deferred
    one s-iteration; ctxB chunk0 same-iteration, chunk1 next iteration.
    Steady-state period ~1.5us/iter with PE/ACT/DVE all ~90% busy.
    PSUM: scA 2 + scB 2 + ctxA 2 + ctxB 2 = 8 banks.
  - Softmax denominators: the ones-column of v_aug makes row 64 of each
    ctx psum the denominator; 1/x runs on a [128,16] reshape (DVE
    reciprocal is ~6 cyc/elem per LANE, so a [1,1024] row costs 6.5us
    but [128,16] costs ~0.1us); the 64-partition broadcast is a log2
    SBUF DMA chain (last block: K=1 PE matmul so the tail never waits).
  - Normalize multiplies run on GpSimd (SBUF-only engine, otherwise
    idle); psum evictions and out-proj drains alternate DVE/ACT.
  - Inputs are DMA'd as full 128-partition tiles (engages all 16 SDMA
    engines) split across the sync and scalar HWDGE queues; q/k tiles
    first so the projections chase the loads.  Output is bf16 (halves
    the tail DMA); bo/bv are applied on the host (linear terms).
"""

import os
import sys

import numpy as np

for _p in ("/opt/trn_rl_repo",):
    if os.path.isdir(_p) and _p not in sys.path:
        sys.path.append(_p)

import ml_dtypes

import concourse.bass as bass
import concourse.mybir as mybir
import concourse.tile as tile
from concourse import bacc
from concourse.bass_utils import run_bass_kernel_spmd

F32 = mybir.dt.float32
BF16 = mybir.dt.bfloat16
I16 = mybir.dt.int16
AF = mybir.ActivationFunctionType
ALU = mybir.AluOpType
BF16_NP = ml_dtypes.bfloat16

D = 1024          # model dim
T = 2048          # query length
S = 2048          # key length
P = 128           # partitions
KT = D // P       # 8 contraction tiles
TT = T // P       # 16 row tiles
ST = S // P       # 16 key tiles
HL = 4            # local heads per core
HD = 64           # head dim
OUTL = HL * HD    # 256 local out dims
VW = HD + 1       # v_aug width per head (ones column appended)
N_CORES = 8

# fast-exp constants: exp(x*0.125) ~= bf16(bitcast(int16(x*EA + EC)))
EA = float(0.125 * 128.0 / np.log(2.0))
EC = float(127 * 128 - 7.5)
I32 = mybir.dt.int32
RMAGIC = 0x7EF311C3   # int-trick reciprocal seed constant


def build_program():
    """Build + compile the SPMD program (same on all 8 cores)."""
    nc = bacc.Bacc(
        "TRN2", target_bir_lowering=False, debug=False, enable_asserts=True,
        num_devices=N_CORES,
    )

    xq_d = nc.dram_tensor("xq", [D, T], BF16, kind="ExternalInput")
    xk_d = nc.dram_tensor("xk", [D, S], BF16, kind="ExternalInput")
    xv_d = nc.dram_tensor("xv", [D, S], BF16, kind="ExternalInput")
    wq_d = nc.dram_tensor("wq", [D, OUTL], BF16, kind="ExternalInput")
    wk_d = nc.dram_tensor("wk", [D, OUTL], BF16, kind="ExternalInput")
    wv_d = nc.dram_tensor("wv", [D, OUTL], BF16, kind="ExternalInput")
    wo_d = nc.dram_tensor("wo", [OUTL, D], BF16, kind="ExternalInput")
    bq_d = nc.dram_tensor("bq", [OUTL, 1], F32, kind="ExternalInput")
    bk_d = nc.dram_tensor("bk", [OUTL, 1], F32, kind="ExternalInput")
    out_d = nc.dram_tensor("out", [T, D], BF16, kind="ExternalOutput")
    wsink_d = nc.dram_tensor("warm_sink", [1, 8], F32, kind="ExternalOutput")

    with tile.TileContext(nc) as tc:
        _build(nc, tc, xq_d, xk_d, xv_d, wq_d, wk_d, wv_d, wo_d,
               bq_d, bk_d, out_d, wsink_d)
    nc.compile()
    return nc


def _build(nc, tc, xq_d, xk_d, xv_d, wq_d, wk_d, wv_d, wo_d,
           bq_d, bk_d, out_d, wsink_d):
    from contextlib import ExitStack

    stack = ExitStack()
    with stack:
        consts = stack.enter_context(tc.tile_pool(name="consts", bufs=1))
        wpool = stack.enter_context(tc.tile_pool(name="wpool", bufs=1))
        acts = stack.enter_context(tc.tile_pool(name="acts", bufs=1))
        stgpool = stack.enter_context(tc.tile_pool(name="stgpool", bufs=2))
        nrmpool = stack.enter_context(tc.tile_pool(name="nrmpool", bufs=1))

        bq_sb = consts.tile([P, 2], F32, name="bq", tag="bq")
        bk_sb = consts.tile([P, 2], F32, name="bk", tag="bk")
        wsnk = consts.tile([1, 8], F32, name="wsnk", tag="wsnk")

        wq_sb = [wpool.tile([P, OUTL], BF16, name=f"wq{k}", tag=f"wq{k}")
                 for k in range(KT)]
        wk_sb = [wpool.tile([P, OUTL], BF16, name=f"wk{k}", tag=f"wk{k}")
                 for k in range(KT)]
        wv_sb = [wpool.tile([P, OUTL], BF16, name=f"wv{k}", tag=f"wv{k}")
                 for k in range(KT)]
        wo_sb = [wpool.tile([P, D], BF16, name=f"wo{k}", tag=f"wo{k}")
                 for k in range(2)]

        qT = [acts.tile([P, T], BF16, name=f"qT{m}", tag=f"qT{m}")
              for m in range(2)]
        kT = [acts.tile([P, S], BF16, name=f"kT{m}", tag=f"kT{m}")
              for m in range(2)]
        v_aug = acts.tile([P, ST * HL * VW], BF16, name="vaug", tag="vaug")
        ctxT = [[acts.tile([P, 1024], BF16, name=f"ctxT{g}{th}",
                           tag=f"ctxT{g}{th}") for th in range(2)]
                for g in range(2)]

        xpool = stack.enter_context(tc.tile_pool(name="xpool", bufs=1))
        xq_sb = [xpool.tile([P, T], BF16, name=f"xq{k}", tag=f"xq{k}")
                 for k in range(KT)]
        xk_sb = [xpool.tile([P, S], BF16, name=f"xk{k}", tag=f"xk{k}")
                 for k in range(KT)]
        xv_sb = [xpool.tile([P, S], BF16, name=f"xv{k}", tag=f"xv{k}")
                 for k in range(KT)]

        # Full 128-partition DMAs (all 16 SDMA engines per transfer),
        # alternating sync/scalar queues; q/k first, v later, wo last.
        nc.sync.dma_start(wq_sb[0][:], wq_d[0:P, :])
        nc.scalar.dma_start(wk_sb[0][:], wk_d[0:P, :])
        nc.sync.dma_start(bq_sb[:], bq_d.rearrange("(m p) o -> p (m o)", p=P))
        nc.scalar.dma_start(bk_sb[:], bk_d.rearrange("(m p) o -> p (m o)",
                                                     p=P))
        nc.sync.dma_start(xq_sb[0][:], xq_d[0:P, :])
        nc.scalar.dma_start(xk_sb[0][:], xk_d[0:P, :])
        for k in range(1, KT):
            nc.sync.dma_start(wq_sb[k][:], wq_d[k * P:(k + 1) * P, :])
            nc.scalar.dma_start(wk_sb[k][:], wk_d[k * P:(k + 1) * P, :])
            nc.sync.dma_start(xq_sb[k][:], xq_d[k * P:(k + 1) * P, :])
            nc.scalar.dma_start(xk_sb[k][:], xk_d[k * P:(k + 1) * P, :])
        for k in range(KT):
            eng = nc.sync if k % 2 == 0 else nc.scalar
            eng.dma_start(wv_sb[k][:], wv_d[k * P:(k + 1) * P, :])
        for k in range(KT):
            eng = nc.sync if k % 2 == 0 else nc.scalar
            eng.dma_start(xv_sb[k][:], xv_d[k * P:(k + 1) * P, :])
        for k in range(2):
            eng = nc.sync if k % 2 == 0 else nc.scalar
            eng.dma_start(wo_sb[k][:], wo_d[k * P:(k + 1) * P, :])

        nc.vector.memset(v_aug[:], 1.0)  # ones columns survive the v writes
        twos = consts.tile([1, 1024], F32, name="twos", tag="twos")
        nc.vector.memset(twos[:], 2.0)

        # ACT exp-table preload during the DMA head
        nc.scalar.activation(wsnk[0:1, 0:2], bq_sb[0:1, 0:2], AF.Exp)

        # ---- q/k projections -------------------------------------------
        with tc.tile_pool(name="qkpsum", bufs=2, space="PSUM") as qkpsum:
            # PE warmup burst (HAM un-throttle) during the DMA head
            warm = qkpsum.tile([P, T], F32, name="pqk", tag="pqk")
            for w in range(16):
                nc.tensor.matmul(warm[:, 0:OUTL], wq_sb[0][:, 0:P],
                                 wq_sb[0][:], start=(w == 0), stop=(w == 15))
            nc.vector.tensor_copy(wsnk[0:1, 2:4], warm[0:1, 0:2])
            nc.sync.dma_start(wsink_d[:, :], wsnk[:])

            for m in range(2):
                for w_sb, x_sb, b_sb, o_sb in (
                        (wq_sb, xq_sb, bq_sb, qT),
                        (wk_sb, xk_sb, bk_sb, kT)):
                    ps = qkpsum.tile([P, T], F32, name="pqk", tag="pqk")
                    for k in range(KT):
                        for c in range(4):
                            cs = slice(c * 512, (c + 1) * 512)
                            nc.tensor.matmul(
                                ps[:, cs], w_sb[k][:, m * P:(m + 1) * P],
                                x_sb[k][:, cs],
                                start=(k == 0), stop=(k == KT - 1))
                    nc.scalar.activation(o_sb[m][:], ps[:], AF.Identity,
                                         bias=b_sb[:, m:m + 1])

        # ---- v projection (no bias: bv is applied on the host) ---------
        with tc.tile_pool(name="vpsum", bufs=2, space="PSUM") as vpsum:
            for s in range(ST):
                ps = vpsum.tile([P, OUTL], F32, name="pv", tag="pv")
                for k in range(KT):
                    nc.tensor.matmul(
                        ps[:], xv_sb[k][:, s * P:(s + 1) * P], wv_sb[k][:],
                        start=(k == 0), stop=(k == KT - 1))
                dst = v_aug[:, s * HL * VW:(s + 1) * HL * VW]
                dst = dst.rearrange("p (h x) -> p h x", x=VW)[:, :, 0:HD]
                nc.scalar.copy(dst, ps[:].rearrange("p (h x) -> p h x", x=HD))

        # ---- attention: 4 blocks of (head pair p, t-half th) -----------
        with tc.tile_pool(name="scpsum", bufs=1, space="PSUM") as scpsum, \
             tc.tile_pool(name="ctxpsum", bufs=1, space="PSUM") as ctxpsum, \
             tc.tile_pool(name="epool", bufs=2) as epool:

            for p in range(2):          # head pair (local heads 2p, 2p+1)
                for th in range(2):     # t halves of 1024
                    t0 = th * 1024
                    ctxA = ctxpsum.tile([VW, 1024], F32, name="ctxA",
                                        tag="ctxA")
                    ctxB = ctxpsum.tile([VW, 1024], F32, name="ctxB",
                                        tag="ctxB")
                    hA = 2 * p
                    hB = 2 * p + 1

                    def vslice(h, s):
                        return slice(s * HL * VW + h * VW,
                                     s * HL * VW + (h + 1) * VW)

                    prevA = None   # (s, exA)
                    prevB = None   # (s, eiB)
                    for s in range(ST):
                        ss = slice(s * P, (s + 1) * P)
                        scA = scpsum.tile([P, 1024], F32, name="scA",
                                          tag="scA")
                        scB = scpsum.tile([P, 1024], F32, name="scB",
                                          tag="scB")
                        for c in range(2):
                            cs = slice(c * 512, (c + 1) * 512)
                            ts_ = slice(t0 + c * 512, t0 + (c + 1) * 512)
                            nc.tensor.matmul(scA[:, cs], kT[p][0:HD, ss],
                                             qT[p][0:HD, ts_],
                                             start=True, stop=True)
                            nc.tensor.matmul(scB[:, cs], kT[p][HD:P, ss],
                                             qT[p][HD:P, ts_],
                                             start=True, stop=True)
                        exA = epool.tile([P, 1024], BF16, name="exA",
                                         tag="exA")
                        eiB = epool.tile([P, 1024], I16, name="eiB",
                                         tag="eiB")
                        # DVE fast-exp, one instruction (per-instr overhead
                        # on the DVE is ~0.4us, so merged beats chunked)
                        nc.vector.tensor_scalar(eiB[:], scB[:],
                                                EA, EC, op0=ALU.mult,
                                                op1=ALU.add)
                        # ACT exact exp, chunked (starts after score chunk0)
                        nc.scalar.activation(exA[:, 0:512], scA[:, 0:512],
                                             AF.Exp, scale=0.125)
                        nc.scalar.activation(exA[:, 512:1024],
                                             scA[:, 512:1024],
                                             AF.Exp, scale=0.125)
                        # deferred ctxA/ctxB for iteration s-1
                        if prevA is not None:
                            sp, pexA = prevA
                            for c in range(2):
                                cs = slice(c * 512, (c + 1) * 512)
                                nc.tensor.matmul(
                                    ctxA[:, cs], v_aug[:, vslice(hA, sp)],
                                    pexA[:, cs],
                                    start=(sp == 0), stop=(sp == ST - 1))
                            sp, peiB = prevB
                            ebB = peiB[:].bitcast(BF16)
                            for c in range(2):
                                cs = slice(c * 512, (c + 1) * 512)
                                nc.tensor.matmul(
                                    ctxB[:, cs], v_aug[:, vslice(hB, sp)],
                                    ebB[:, cs],
                                    start=(sp == 0), stop=(sp == ST - 1))
                        prevA = (s, exA)
                        prevB = (s, eiB)
                    # drain the deferred tails
                    sp, pexA = prevA
                    sp2, peiB = prevB
                    for c in range(2):
                        cs = slice(c * 512, (c + 1) * 512)
                        nc.tensor.matmul(ctxA[:, cs],
                                         v_aug[:, vslice(hA, sp)],
                                         pexA[:, cs],
                                         start=(sp == 0), stop=True)
                    ebB = peiB[:].bitcast(BF16)
                    for c in range(2):
                        cs = slice(c * 512, (c + 1) * 512)
                        nc.tensor.matmul(ctxB[:, cs],
                                         v_aug[:, vslice(hB, sp2)],
                                         ebB[:, cs],
                                         start=(sp2 == 0), stop=True)

                    # evict ctx (DVE + ACT) and denom rows (DVE, to base
                    # partition 0); 1/denom = int-trick seed (DVE) + one
                    # Newton step; broadcast + normalize on GpSimd for
                    # blocks 0-2 (off the critical path), on the
                    # then-idle DVE for the last block.
                    last = (p, th) == (1, 1)
                    stgA = stgpool.tile([HD, 1024], F32, name="stgA",
                                        tag="stgA")
                    stgB = stgpool.tile([HD, 1024], F32, name="stgB",
                                        tag="stgB")
                    nc.vector.tensor_copy(stgA[:], ctxA[0:HD, :])
                    nc.scalar.copy(stgB[:], ctxB[0:HD, :])
                    for i, (ctx, stg) in ((0, (ctxA, stgA)),
                                          (1, (ctxB, stgB))):
                        eng = nc.vector if last else nc.gpsimd
                        drow = nrmpool.tile([1, 1024], F32, name=f"dr{i}",
                                            tag="dr")
                        nc.vector.tensor_copy(drow[:], ctx[HD:HD + 1, :])
                        seed = nrmpool.tile([1, 1024], I32, name=f"sd{i}",
                                            tag="sd")
                        nc.vector.tensor_scalar(
                            seed[:], drow[:].bitcast(I32), -1, RMAGIC,
                            op0=ALU.mult, op1=ALU.add)
                        r0f = seed[:].bitcast(F32)
                        e = nrmpool.tile([1, 1024], F32, name=f"e{i}",
                                         tag="e")
                        eng.tensor_tensor(out=e[:], in0=drow[:],
                                          in1=r0f, op=ALU.mult)
                        t_ = nrmpool.tile([1, 1024], F32, name=f"t{i}",
                                          tag="t")
                        eng.tensor_tensor(out=t_[:], in0=twos[:],
                                          in1=e[:], op=ALU.subtract)
                        rn = nrmpool.tile([1, 1024], F32, name=f"rn{i}",
                                          tag=f"rn{i}")
                        eng.tensor_tensor(out=rn[:], in0=r0f,
                                          in1=t_[:], op=ALU.mult)
                        rb = nrmpool.tile([HD, 1024], F32,
                                          name=f"rb{i}", tag="rb")
                        nc.gpsimd.partition_broadcast(rb[:], rn[:])
                        if i == 0:
                            eng.tensor_tensor(
                                out=ctxT[p][th][0:HD, :],
                                in0=stg[:], in1=rb[:],
                                op=ALU.mult)
                        else:
                            ostg = nrmpool.tile([HD, 1024], BF16,
                                                name="ostg", tag="ostg")
                            eng.tensor_tensor(
                                out=ostg[:], in0=stg[:],
                                in1=rb[:], op=ALU.mult)
                            nc.scalar.dma_start(
                                ctxT[p][th][HD:P, :], ostg[:])

        # ---- output projection -----------------------------------------
        with tc.tile_pool(name="popsum", bufs=2, space="PSUM") as popsum, \
             tc.tile_pool(name="opool", bufs=3) as opool:

            def emit_outproj(trange):
                for t in trange:
                    th_, tt_ = divmod(t, TT // 2)
                    ts_ = slice(tt_ * P, (tt_ + 1) * P)
                    po = popsum.tile([P, D], F32, name="po", tag="po")
                    for g in range(2):
                        for n in range(2):
                            ns = slice(n * 512, (n + 1) * 512)
                            nc.tensor.matmul(po[:, ns],
                                             ctxT[g][th_][:, ts_],
                                             wo_sb[g][:, ns],
                                             start=(g == 0), stop=(g == 1))
                    ost = opool.tile([P, D], BF16, name="ost", tag="ost")
                    if t % 2 == 0:
                        nc.vector.tensor_copy(ost[:], po[:])
                    else:
                        nc.scalar.copy(ost[:], po[:])
                    nc.sync.dma_start(out_d[t * P:(t + 1) * P, :], ost[:])

            emit_outproj(range(0, TT))


def make_in_maps(query, key, value, Wq, bq, Wk, bk, Wv, bv, Wo, bo):
    """Shard the full inputs into the 8 per-core input dicts."""
    query, key, value, Wq, bq, Wk, bk, Wv, bv, Wo, bo = [
        np.asarray(a, dtype=np.float32)
        for a in (query, key, value, Wq, bq, Wk, bk, Wv, bv, Wo, bo)]

    def bf(a):
        return np.ascontiguousarray(a).astype(BF16_NP)

    in_maps = []
    for c in range(N_CORES):
        b, g = divmod(c, 4)
        sl = slice(g * OUTL, (g + 1) * OUTL)
        in_maps.append({
            "xq": bf(query[b].T),
            "xk": bf(key[b].T),
            "xv": bf(value[b].T),
            "wq": bf(Wq[sl, :].T),
            "wk": bf(Wk[sl, :].T),
            "wv": bf(Wv[sl, :].T),
            "wo": bf(Wo[:, sl].T),
            "bq": np.ascontiguousarray(bq[sl].reshape(OUTL, 1)),
            "bk": np.ascontiguousarray(bk[sl].reshape(OUTL, 1)),
        })
    return in_maps


def gather_out(results, Wo, bo, bv):
    """Sum the per-core bf16 partials and add the host-side bias terms."""
    Wo = np.asarray(Wo, np.float32)
    bo = np.asarray(bo, np.float32)
    bv = np.asarray(bv, np.float32)
    host_bias = bo + bv @ Wo.T
    out = np.empty((2, T, D), dtype=np.float32)
    for b in range(2):
        acc = results[4 * b]["out"].astype(np.float32)
        for g in range(1, 4):
            acc = acc + results[4 * b + g]["out"].astype(np.float32)
        out[b] = acc + host_bias
    return out


_NC_CACHE = None


def _get_nc():
    global _NC_CACHE
    if _NC_CACHE is None:
        _NC_CACHE = build_program()
    return _NC_CACHE


def kernel(query, key, value, Wq, bq, Wk, bk, Wv, bv, Wo, bo):
    nc = _get_nc()
    in_maps = make_in_maps(query, key, value, Wq, bq, Wk, bk, Wv, bv, Wo, bo)
    res = run_bass_kernel_spmd(nc, in_maps, list(range(N_CORES))).results
    return gather_out(res, Wo, bo, bv)


# revision 13
# speedup vs baseline: 1.6124x; 1.2443x over previous
"""Bass/Tile TRN2 kernel for nn_MultiHeadAttention_9277129359942.

B=2, T=S=2048, D=1024, H=16 heads, head_dim=64, fp32 I/O.

Sharding (8 cores): data-parallel over batch (2) x tensor-parallel over
head groups (4 heads / core, 256 out dims).  Each core computes the
attention for its 4 heads and a partial output projection; the host sums
the 4 bf16 partials per batch and adds the (linear) bo and bv terms
exactly: out = sum_g partial_g + bo + bv @ Wo.T.

v3 design notes:
  - Softmax exp is split across engines: head A of each pair uses the
    exact ACT exp (2 x N=512 chunks, started as soon as each score
    chunk lands), head B uses a one-instruction DVE fast-exp
    (Schraudolph: int16(x*EA+EC) bitcast as bf16, ~4% max rel err).
    End-to-end rel err 1.39e-2 (gate 2e-2), verified vs the reference.
  - Software pipelining in the attention loop: ctxA (ACT head) deferred
    one s-iteration; ctxB chunk0 same-iteration, chunk1 next iteration.
    Steady-state period ~1.5us/iter with PE/ACT/DVE all ~90% busy.
    PSUM: scA 2 + scB 2 + ctxA 2 + ctxB 2 = 8 banks.
  - Softmax denominators: the ones-column of v_aug makes row 64 of each
    ctx psum the denominator; 1/x runs on a [128,16] reshape (DVE
    reciprocal is ~6 cyc/elem per LANE, so a [1,1024] row costs 6.5us
    but [128,16] costs ~0.1us); the 64-partition broadcast is a log2
    SBUF DMA chain (last block: K=1 PE matmul so the tail never waits).
  - Normalize multiplies run on GpSimd (SBUF-only engine, otherwise
    idle); psum evictions and out-proj drains alternate DVE/ACT.
  - Inputs are DMA'd as full 128-partition tiles (engages all 16 SDMA
    engines) split across the sync and scalar HWDGE queues; q/k tiles
    first so the projections chase the loads.  Output is bf16 (halves
    the tail DMA); bo/bv are applied on the host (linear terms).
"""

import os
import sys

import numpy as np

for _p in ("/opt/trn_rl_repo",):
    if os.path.isdir(_p) and _p not in sys.path:
        sys.path.append(_p)

import ml_dtypes

import concourse.bass as bass
import concourse.mybir as mybir
import concourse.tile as tile
from concourse import bacc
from concourse.bass_utils import run_bass_kernel_spmd

F32 = mybir.dt.float32
BF16 = mybir.dt.bfloat16
I16 = mybir.dt.int16
AF = mybir.ActivationFunctionType
ALU = mybir.AluOpType
BF16_NP = ml_dtypes.bfloat16

D = 1024          # model dim
T = 2048          # query length
S = 2048          # key length
P = 128           # partitions
KT = D // P       # 8 contraction tiles
TT = T // P       # 16 row tiles
ST = S // P       # 16 key tiles
HL = 4            # local heads per core
HD = 64           # head dim
OUTL = HL * HD    # 256 local out dims
VW = HD + 1       # v_aug width per head (ones column appended)
N_CORES = 8

# fast-exp constants: exp(x*0.125) ~= bf16(bitcast(int16(x*EA + EC)))
EA = float(0.125 * 128.0 / np.log(2.0))
EC = float(127 * 128 - 7.5)
I32 = mybir.dt.int32
RMAGIC = 0x7EF311C3   # int-trick reciprocal seed constant


def build_program():
    """Build + compile the SPMD program (same on all 8 cores)."""
    nc = bacc.Bacc(
        "TRN2", target_bir_lowering=False, debug=False, enable_asserts=True,
        num_devices=N_CORES,
    )

    xq_d = nc.dram_tensor("xq", [D, T], BF16, kind="ExternalInput")
    xk_d = nc.dram_tensor("xk", [D, S], BF16, kind="ExternalInput")
    xv_d = nc.dram_tensor("xv", [D, S], BF16, kind="ExternalInput")
    wq_d = nc.dram_tensor("wq", [D, OUTL], BF16, kind="ExternalInput")
    wk_d = nc.dram_tensor("wk", [D, OUTL], BF16, kind="ExternalInput")
    wv_d = nc.dram_tensor("wv", [D, OUTL], BF16, kind="ExternalInput")
    wo_d = nc.dram_tensor("wo", [OUTL, D], BF16, kind="ExternalInput")
    bq_d = nc.dram_tensor("bq", [OUTL, 1], F32, kind="ExternalInput")
    bk_d = nc.dram_tensor("bk", [OUTL, 1], F32, kind="ExternalInput")
    out_d = nc.dram_tensor("out", [T, D], BF16, kind="ExternalOutput")
    wsink_d = nc.dram_tensor("warm_sink", [1, 8], F32, kind="ExternalOutput")

    with tile.TileContext(nc) as tc:
        _build(nc, tc, xq_d, xk_d, xv_d, wq_d, wk_d, wv_d, wo_d,
               bq_d, bk_d, out_d, wsink_d)
    nc.compile()
    return nc


def _build(nc, tc, xq_d, xk_d, xv_d, wq_d, wk_d, wv_d, wo_d,
           bq_d, bk_d, out_d, wsink_d):
    from contextlib import ExitStack

    stack = ExitStack()
    with stack:
        consts = stack.enter_context(tc.tile_pool(name="consts", bufs=1))
        wpool = stack.enter_context(tc.tile_pool(name="wpool", bufs=1))
        acts = stack.enter_context(tc.tile_pool(name="acts", bufs=1))


        bq_sb = consts.tile([P, 2], F32, name="bq", tag="bq")
        bk_sb = consts.tile([P, 2], F32, name="bk", tag="bk")
        wsnk = consts.tile([1, 8], F32, name="wsnk", tag="wsnk")

        wq_sb = [wpool.tile([P, OUTL], BF16, name=f"wq{k}", tag=f"wq{k}")
                 for k in range(KT)]
        wk_sb = [wpool.tile([P, OUTL], BF16, name=f"wk{k}", tag=f"wk{k}")
                 for k in range(KT)]
        wv_sb = [wpool.tile([P, OUTL], BF16, name=f"wv{k}", tag=f"wv{k}")
                 for k in range(KT)]
        wo_sb = [wpool.tile([P, D], BF16, name=f"wo{k}", tag=f"wo{k}")
                 for k in range(2)]

        qT = [acts.tile([P, T], BF16, name=f"qT{m}", tag=f"qT{m}")
              for m in range(2)]
        kT = [acts.tile([P, S], BF16, name=f"kT{m}", tag=f"kT{m}")
              for m in range(2)]
        v_aug = acts.tile([P, ST * HL * VW], BF16, name="vaug", tag="vaug")
        ctxT = [[acts.tile([P, 1024], BF16, name=f"ctxT{g}{th}",
                           tag=f"ctxT{g}{th}") for th in range(2)]
                for g in range(2)]

        xpool_cm = tc.tile_pool(name="xpool", bufs=1)
        xpool = xpool_cm.__enter__()
        xq_sb = [xpool.tile([P, T], BF16, name=f"xq{k}", tag=f"xq{k}")
                 for k in range(KT)]
        xk_sb = [xpool.tile([P, S], BF16, name=f"xk{k}", tag=f"xk{k}")
                 for k in range(KT)]
        xv_sb = [xpool.tile([P, S], BF16, name=f"xv{k}", tag=f"xv{k}")
                 for k in range(KT)]

        # Full 128-partition DMAs (all 16 SDMA engines per transfer),
        # alternating sync/scalar queues; q/k first, v later, wo last.
        nc.sync.dma_start(wq_sb[0][:], wq_d[0:P, :])
        nc.scalar.dma_start(wk_sb[0][:], wk_d[0:P, :])
        nc.sync.dma_start(bq_sb[:], bq_d.rearrange("(m p) o -> p (m o)", p=P))
        nc.scalar.dma_start(bk_sb[:], bk_d.rearrange("(m p) o -> p (m o)",
                                                     p=P))
        nc.sync.dma_start(xq_sb[0][:], xq_d[0:P, :])
        nc.scalar.dma_start(xk_sb[0][:], xk_d[0:P, :])
        for k in range(1, KT):
            nc.sync.dma_start(wq_sb[k][:], wq_d[k * P:(k + 1) * P, :])
            nc.scalar.dma_start(wk_sb[k][:], wk_d[k * P:(k + 1) * P, :])
            nc.sync.dma_start(xq_sb[k][:], xq_d[k * P:(k + 1) * P, :])
            nc.scalar.dma_start(xk_sb[k][:], xk_d[k * P:(k + 1) * P, :])
        for k in range(KT):
            eng = nc.sync if k % 2 == 0 else nc.scalar
            eng.dma_start(wv_sb[k][:], wv_d[k * P:(k + 1) * P, :])
        for k in range(KT):
            eng = nc.sync if k % 2 == 0 else nc.scalar
            eng.dma_start(xv_sb[k][:], xv_d[k * P:(k + 1) * P, :])
        for k in range(2):
            eng = nc.sync if k % 2 == 0 else nc.scalar
            eng.dma_start(wo_sb[k][:], wo_d[k * P:(k + 1) * P, :])

        nc.vector.memset(v_aug[:], 1.0)  # ones columns survive the v writes
        twos = consts.tile([1, 1024], F32, name="twos", tag="twos")
        nc.vector.memset(twos[:], 2.0)

        # ACT exp-table preload during the DMA head
        nc.scalar.activation(wsnk[0:1, 0:2], bq_sb[0:1, 0:2], AF.Exp)

        # ---- q/k projections: k-outer (chases the x DMAs) over two
        # t-half passes; all four (m, q/k) psum groups live at once -----
        with tc.tile_pool(name="qkpsum", bufs=1, space="PSUM") as qkpsum:
            # PE warmup burst (HAM un-throttle) during the DMA head
            warm = qkpsum.tile([P, 1024], F32, name="pq00", tag="pq0")
            for w in range(16):
                nc.tensor.matmul(warm[:, 0:OUTL], wq_sb[0][:, 0:P],
                                 wq_sb[0][:], start=(w == 0), stop=(w == 15))
            nc.vector.tensor_copy(wsnk[0:1, 2:4], warm[0:1, 0:2])
            nc.sync.dma_start(wsink_d[:, :], wsnk[:])

            groups = [(m, w_sb, x_sb, b_sb, o_sb)
                      for m in range(2)
                      for w_sb, x_sb, b_sb, o_sb in
                      ((wq_sb, xq_sb, bq_sb, qT),
                       (wk_sb, xk_sb, bk_sb, kT))]
            for tg in range(2):
                t_lo = tg * 1024
                ps_g = [qkpsum.tile([P, 1024], F32, name=f"pq{gi}{tg}",
                                    tag=f"pq{gi}")
                        for gi in range(4)]
                for k in range(KT):
                    for gi, (m, w_sb, x_sb, b_sb, o_sb) in enumerate(groups):
                        for c in range(2):
                            cs = slice(c * 512, (c + 1) * 512)
                            xs = slice(t_lo + c * 512, t_lo + (c + 1) * 512)
                            nc.tensor.matmul(
                                ps_g[gi][:, cs],
                                w_sb[k][:, m * P:(m + 1) * P],
                                x_sb[k][:, xs],
                                start=(k == 0), stop=(k == KT - 1))
                for gi, (m, w_sb, x_sb, b_sb, o_sb) in enumerate(groups):
                    nc.scalar.activation(
                        o_sb[m][:, t_lo:t_lo + 1024], ps_g[gi][:],
                        AF.Identity, bias=b_sb[:, m:m + 1])

        # ---- v projection (no bias: bv is applied on the host) ---------
        with tc.tile_pool(name="vpsum", bufs=2, space="PSUM") as vpsum:
            for s in range(ST):
                ps = vpsum.tile([P, OUTL], F32, name="pv", tag="pv")
                for k in range(KT):
                    nc.tensor.matmul(
                        ps[:], xv_sb[k][:, s * P:(s + 1) * P], wv_sb[k][:],
                        start=(k == 0), stop=(k == KT - 1))
                dst = v_aug[:, s * HL * VW:(s + 1) * HL * VW]
                dst = dst.rearrange("p (h x) -> p h x", x=VW)[:, :, 0:HD]
                nc.scalar.copy(dst, ps[:].rearrange("p (h x) -> p h x", x=HD))
        xpool_cm.__exit__(None, None, None)

        # ---- attention: 4 blocks of (head pair p, t-half th) -----------
        with tc.tile_pool(name="scpsum", bufs=1, space="PSUM") as scpsum, \
             tc.tile_pool(name="ctxpsum", bufs=1, space="PSUM") as ctxpsum, \
             tc.tile_pool(name="stgpool", bufs=2) as stgpool, \
             tc.tile_pool(name="nrmpool", bufs=2) as nrmpool, \
             tc.tile_pool(name="epool", bufs=2) as epool:

            for p in range(2):          # head pair (local heads 2p, 2p+1)
                for th in range(2):     # t halves of 1024
                    t0 = th * 1024
                    ctxA = ctxpsum.tile([VW, 1024], F32, name="ctxA",
                                        tag="ctxA")
                    ctxB = ctxpsum.tile([VW, 1024], F32, name="ctxB",
                                        tag="ctxB")
                    hA = 2 * p
                    hB = 2 * p + 1

                    def vslice(h, s):
                        return slice(s * HL * VW + h * VW,
                                     s * HL * VW + (h + 1) * VW)

                    prevA = None   # (s, exA)
                    prevB = None   # (s, eiB)
                    for s in range(ST):
                        ss = slice(s * P, (s + 1) * P)
                        scA = scpsum.tile([P, 1024], F32, name="scA",
                                          tag="scA")
                        scB = scpsum.tile([P, 1024], F32, name="scB",
                                          tag="scB")
                        for c in range(2):
                            cs = slice(c * 512, (c + 1) * 512)
                            ts_ = slice(t0 + c * 512, t0 + (c + 1) * 512)
                            nc.tensor.matmul(scA[:, cs], kT[p][0:HD, ss],
                                             qT[p][0:HD, ts_],
                                             start=True, stop=True)
                            nc.tensor.matmul(scB[:, cs], kT[p][HD:P, ss],
                                             qT[p][HD:P, ts_],
                                             start=True, stop=True)
                        exA = epool.tile([P, 1024], BF16, name="exA",
                                         tag="exA")
                        eiB = epool.tile([P, 1024], I16, name="eiB",
                                         tag="eiB")
                        # DVE fast-exp, one instruction (per-instr overhead
                        # on the DVE is ~0.4us, so merged beats chunked)
                        nc.vector.tensor_scalar(eiB[:], scB[:],
                                                EA, EC, op0=ALU.mult,
                                                op1=ALU.add)
                        # ACT exact exp, chunked (starts after score chunk0)
                        nc.scalar.activation(exA[:, 0:512], scA[:, 0:512],
                                             AF.Exp, scale=0.125)
                        nc.scalar.activation(exA[:, 512:1024],
                                             scA[:, 512:1024],
                                             AF.Exp, scale=0.125)
                        # deferred ctxA/ctxB for iteration s-1
                        if prevA is not None:
                            sp, pexA = prevA
                            for c in range(2):
                                cs = slice(c * 512, (c + 1) * 512)
                                nc.tensor.matmul(
                                    ctxA[:, cs], v_aug[:, vslice(hA, sp)],
                                    pexA[:, cs],
                                    start=(sp == 0), stop=(sp == ST - 1))
                            sp, peiB = prevB
                            ebB = peiB[:].bitcast(BF16)
                            for c in range(2):
                                cs = slice(c * 512, (c + 1) * 512)
                                nc.tensor.matmul(
                                    ctxB[:, cs], v_aug[:, vslice(hB, sp)],
                                    ebB[:, cs],
                                    start=(sp == 0), stop=(sp == ST - 1))
                        prevA = (s, exA)
                        prevB = (s, eiB)
                    # drain the deferred tails
                    sp, pexA = prevA
                    sp2, peiB = prevB
                    for c in range(2):
                        cs = slice(c * 512, (c + 1) * 512)
                        nc.tensor.matmul(ctxA[:, cs],
                                         v_aug[:, vslice(hA, sp)],
                                         pexA[:, cs],
                                         start=(sp == 0), stop=True)
                    ebB = peiB[:].bitcast(BF16)
                    for c in range(2):
                        cs = slice(c * 512, (c + 1) * 512)
                        nc.tensor.matmul(ctxB[:, cs],
                                         v_aug[:, vslice(hB, sp2)],
                                         ebB[:, cs],
                                         start=(sp2 == 0), stop=True)

                    # evict ctx (DVE + ACT) and denom rows (DVE, to base
                    # partition 0); 1/denom = int-trick seed (DVE) + one
                    # Newton step; broadcast + normalize on GpSimd for
                    # blocks 0-2 (off the critical path), on the
                    # then-idle DVE for the last block.
                    last = (p, th) == (1, 1)
                    stgA = stgpool.tile([HD, 1024], F32, name="stgA",
                                        tag="stgA")
                    stgB = stgpool.tile([HD, 1024], F32, name="stgB",
                                        tag="stgB")
                    nc.vector.tensor_copy(stgA[:], ctxA[0:HD, :])
                    nc.scalar.copy(stgB[:], ctxB[0:HD, :])
                    for i, (ctx, stg) in ((0, (ctxA, stgA)),
                                          (1, (ctxB, stgB))):
                        eng = nc.vector if last else nc.gpsimd
                        drow = nrmpool.tile([1, 1024], F32, name=f"dr{i}",
                                            tag=f"dr{i}")
                        nc.vector.tensor_copy(drow[:], ctx[HD:HD + 1, :])
                        seed = nrmpool.tile([1, 1024], I32, name=f"sd{i}",
                                            tag=f"sd{i}")
                        nc.vector.tensor_scalar(
                            seed[:], drow[:].bitcast(I32), -1, RMAGIC,
                            op0=ALU.mult, op1=ALU.add)
                        r0f = seed[:].bitcast(F32)
                        e = nrmpool.tile([1, 1024], F32, name=f"e{i}",
                                         tag=f"e{i}")
                        eng.tensor_tensor(out=e[:], in0=drow[:],
                                          in1=r0f, op=ALU.mult)
                        t_ = nrmpool.tile([1, 1024], F32, name=f"t{i}",
                                          tag=f"t{i}")
                        eng.tensor_tensor(out=t_[:], in0=twos[:],
                                          in1=e[:], op=ALU.subtract)
                        rn = nrmpool.tile([1, 1024], F32, name=f"rn{i}",
                                          tag=f"rn{i}")
                        eng.tensor_tensor(out=rn[:], in0=r0f,
                                          in1=t_[:], op=ALU.mult)
                        rb = nrmpool.tile([HD, 1024], F32,
                                          name=f"rb{i}", tag=f"rb{i}")
                        nc.gpsimd.partition_broadcast(rb[:], rn[:])
                        if i == 0:
                            eng.tensor_tensor(
                                out=ctxT[p][th][0:HD, :],
                                in0=stg[:], in1=rb[:],
                                op=ALU.mult)
                        else:
                            ostg = nrmpool.tile([HD, 1024], BF16,
                                                name="ostg", tag="ostg")
                            eng.tensor_tensor(
                                out=ostg[:], in0=stg[:],
                                in1=rb[:], op=ALU.mult)
                            nc.scalar.dma_start(
                                ctxT[p][th][HD:P, :], ostg[:])

        # ---- output projection -----------------------------------------
        with tc.tile_pool(name="popsum", bufs=2, space="PSUM") as popsum, \
             tc.tile_pool(name="opool", bufs=3) as opool:

            def emit_outproj(trange):
                for t in trange:
                    th_, tt_ = divmod(t, TT // 2)
                    ts_ = slice(tt_ * P, (tt_ + 1) * P)
                    po = popsum.tile([P, D], F32, name="po", tag="po")
                    for g in range(2):
                        for n in range(2):
                            ns = slice(n * 512, (n + 1) * 512)
                            nc.tensor.matmul(po[:, ns],
                                             ctxT[g][th_][:, ts_],
                                             wo_sb[g][:, ns],
                                             start=(g == 0), stop=(g == 1))
                    ost = opool.tile([P, D], BF16, name="ost", tag="ost")
                    if t % 2 == 0:
                        nc.vector.tensor_copy(ost[:], po[:])
                    else:
                        nc.scalar.copy(ost[:], po[:])
                    nc.sync.dma_start(out_d[t * P:(t + 1) * P, :], ost[:])

            emit_outproj(range(0, TT))


def make_in_maps(query, key, value, Wq, bq, Wk, bk, Wv, bv, Wo, bo):
    """Shard the full inputs into the 8 per-core input dicts."""
    query, key, value, Wq, bq, Wk, bk, Wv, bv, Wo, bo = [
        np.asarray(a, dtype=np.float32)
        for a in (query, key, value, Wq, bq, Wk, bk, Wv, bv, Wo, bo)]

    def bf(a):
        return np.ascontiguousarray(a).astype(BF16_NP)

    in_maps = []
    for c in range(N_CORES):
        b, g = divmod(c, 4)
        sl = slice(g * OUTL, (g + 1) * OUTL)
        in_maps.append({
            "xq": bf(query[b].T),
            "xk": bf(key[b].T),
            "xv": bf(value[b].T),
            "wq": bf(Wq[sl, :].T),
            "wk": bf(Wk[sl, :].T),
            "wv": bf(Wv[sl, :].T),
            "wo": bf(Wo[:, sl].T),
            "bq": np.ascontiguousarray(bq[sl].reshape(OUTL, 1)),
            "bk": np.ascontiguousarray(bk[sl].reshape(OUTL, 1)),
        })
    return in_maps


def gather_out(results, Wo, bo, bv):
    """Sum the per-core bf16 partials and add the host-side bias terms."""
    Wo = np.asarray(Wo, np.float32)
    bo = np.asarray(bo, np.float32)
    bv = np.asarray(bv, np.float32)
    host_bias = bo + bv @ Wo.T
    out = np.empty((2, T, D), dtype=np.float32)
    for b in range(2):
        acc = results[4 * b]["out"].astype(np.float32)
        for g in range(1, 4):
            acc = acc + results[4 * b + g]["out"].astype(np.float32)
        out[b] = acc + host_bias
    return out


_NC_CACHE = None


def _get_nc():
    global _NC_CACHE
    if _NC_CACHE is None:
        _NC_CACHE = build_program()
    return _NC_CACHE


def kernel(query, key, value, Wq, bq, Wk, bk, Wv, bv, Wo, bo):
    nc = _get_nc()
    in_maps = make_in_maps(query, key, value, Wq, bq, Wk, bk, Wv, bv, Wo, bo)
    res = run_bass_kernel_spmd(nc, in_maps, list(range(N_CORES))).results
    return gather_out(res, Wo, bo, bv)
